# revision 20
# baseline (speedup 1.0000x reference)
"""GCN message-passing layer (4x GCNConv sum) on 8 Trainium2 NeuronCores.

out[d] = sum_i ( segment_sum_{e in E_i, dst=d} x[src_e] ) @ W_i

Self-contained kernel: takes FULL inputs, shards destination nodes across the
8 cores (graph parallel, x replicated), runs one SPMD Bass/Tile program via
run_bass_kernel_spmd, returns the FULL [N, H] output.

Per-core pipeline (no collectives needed):
  - Host groups edges by (core, set, dst-tile(256), src-bank(32768)); each
    group is padded to a cross-core-uniform multiple of 128 so one program
    fits every core's data (pad: src=0, dstf=-1).
  - dma_gather (int16 bank-local idxs, single_packet=False) stages x[src]
    rows (bf16) into SBUF; calls are 96 staging columns (12288 rows) each and
    cycle over 4 SWDGE queues (num_swdge_queues=4) — SWDGE desc-gen on the
    Pool engine is the kernel's wall (~4 ns/row single-queue floor ~8).
  - One-hot S[e, d] = (dstf[e] == d) matrices are built in BULK: a single
    VectorE tensor_tensor is_equal per (set, span) with stride-0 broadcast
    APs (dstf [P,cols,1] vs iota [P,1,256]) — replacing per-chunk
    tensor_scalar ops whose per-chunk handshakes paced the whole pipeline.
  - Each 128-edge chunk is segment-summed into PSUM by TensorE
    (lhsT=msg chunk, rhs=S column block); pad edges select no column.
    gpsum[t] holds g_i^T = [128h, 256d] for the 4 edge sets.
  - Phase 2: out[d, :] += (g_i^T).T @ W_i in fp32, accumulated over the 4
    sets in PSUM, then DMA'd to the per-core output rows.
"""
import math
import sys

sys.path.insert(0, "/opt/trn_rl_repo")

import numpy as np
import ml_dtypes

from concourse import bass, mybir, tile, bacc
from concourse.bass_utils import run_bass_kernel_spmd

P = 128
N_CORES = 8
DT_TILE = 256
BANK_ROWS = 32768
SPAN = 2
N_SETS = 4


class _Cfg:
    def __init__(self, n_nodes):
        self.n_nodes = n_nodes
        self.npc = n_nodes // N_CORES
        self.nt = math.ceil(self.npc / DT_TILE)
        self.n_banks = math.ceil(n_nodes / BANK_ROWS)
        self.x_rows_pad = self.n_banks * BANK_ROWS
        self.n_spans = math.ceil(self.nt / SPAN)


def _host_prep(cfg, edges_list):
    NC, NS, NT, NB = N_CORES, N_SETS, cfg.nt, cfg.n_banks
    counts = np.zeros((NC, NS, NT, NB), np.int64)
    per_set = []
    for i, e in enumerate(edges_list):
        src = np.asarray(e[0], np.int64)
        dst = np.asarray(e[1], np.int64)
        core = dst // cfg.npc
        dloc = dst % cfg.npc
        t = dloc // DT_TILE
        b = src // BANK_ROWS
        key = (core * NT + t) * NB + b
        counts[:, i] = np.bincount(key, minlength=NC * NT * NB).reshape(NC, NT, NB)
        order = np.argsort(key, kind="stable")
        per_set.append((src[order], dloc[order], key[order]))

    C = -(-counts.max(axis=0) // P)

    col_of = np.zeros((NS, NT, NB), np.int64)
    unit_cols = {}
    col = 0
    for s in range(cfg.n_spans):
        ts = range(s * SPAN, min((s + 1) * SPAN, NT))
        for i in range(NS):
            for b in range(NB):
                for t in ts:
                    col_of[i, t, b] = col
                    col += C[i, t, b]
                unit_cols[(s, i, b)] = int(C[i, list(ts), b].sum())
    totc = col
    tot_slots = totc * P

    idx_mats, dstf_mats = [], []
    for c in range(NC):
        slot_src = np.zeros(tot_slots, np.int64)
        slot_dstf = np.full(tot_slots, -1.0, np.float32)
        for i in range(NS):
            src_s, dloc_s, key_s = per_set[i]
            lo = np.searchsorted(key_s, c * NT * NB)
            hi = np.searchsorted(key_s, (c + 1) * NT * NB)
            src_c, dloc_c, key_c = src_s[lo:hi], dloc_s[lo:hi], key_s[lo:hi]
            t_c = (key_c // NB) % NT
            b_c = key_c % NB
            gstart = np.searchsorted(key_c, key_c)
            rank = np.arange(len(key_c)) - gstart
            slot = col_of[i, t_c, b_c] * P + rank
            slot_src[slot] = src_c - b_c * BANK_ROWS
            slot_dstf[slot] = (dloc_c - t_c * DT_TILE).astype(np.float32)
        idx16 = slot_src.reshape(tot_slots // 16, 16).T.astype(np.int16)
        idx_mats.append(np.tile(idx16, (8, 1)))
        dstf_mats.append(slot_dstf.reshape(totc, P).T.astype(ml_dtypes.bfloat16))

    return dict(C=C, col_of=col_of, unit_cols=unit_cols, totc=totc,
                idx_mats=idx_mats, dstf_mats=dstf_mats)


def _first_b(C, i, t):
    for b in range(C.shape[2]):
        if C[i, t, b] > 0:
            return b
    return -1


def _last_b(C, i, t):
    for b in range(C.shape[2] - 1, -1, -1):
        if C[i, t, b] > 0:
            return b
    return -1


def _build_kernel(cfg, prep):
    NS, NT, NB = N_SETS, cfg.nt, cfg.n_banks
    C, col_of, unit_cols, totc = (prep["C"], prep["col_of"], prep["unit_cols"],
                                  prep["totc"])
    DT = DT_TILE
    msg_dt = mybir.dt.bfloat16
    s_dt = msg_dt

    nc = bacc.Bacc("TRN2", target_bir_lowering=False, debug=False,
                   num_devices=N_CORES, num_swdge_queues=4)
    gq = [0]
    x = nc.dram_tensor("x", [cfg.x_rows_pad, P], msg_dt, kind="ExternalInput").ap()
    idx_d = nc.dram_tensor("idx", [P, totc * 8], mybir.dt.int16, kind="ExternalInput").ap()
    dstf_d = nc.dram_tensor("dstf", [P, totc], mybir.dt.bfloat16, kind="ExternalInput").ap()
    iota_d = nc.dram_tensor("iota", [P, DT], s_dt, kind="ExternalInput").ap()
    w_d = nc.dram_tensor("w", [NS * P, P], mybir.dt.float32, kind="ExternalInput").ap()
    out_d = nc.dram_tensor("out", [cfg.npc, P], mybir.dt.float32, kind="ExternalOutput").ap()

    with tile.TileContext(nc) as tc:
        with tc.tile_pool(name="const", bufs=1) as constp, \
             tc.tile_pool(name="idxp", bufs=3) as idxp, \
             tc.tile_pool(name="dstfp", bufs=3) as dstfp, \
             tc.tile_pool(name="stg", bufs=2) as stgp, \
             tc.tile_pool(name="sp", bufs=2) as spool, \
             tc.tile_pool(name="gsb", bufs=3) as gsbp, \
             tc.tile_pool(name="osb", bufs=4) as osbp, \
             tc.tile_pool(name="gps", bufs=2, space="PSUM") as gpsp, \
             tc.tile_pool(name="ops", bufs=2, space="PSUM") as opsp:

            iota_sb = constp.tile([P, 1, DT], s_dt)
            nc.sync.dma_start(out=iota_sb[:, 0, :], in_=iota_d[:])
            w_sb = constp.tile([P, NS, P], mybir.dt.float32)
            for i in range(NS):
                nc.sync.dma_start(out=w_sb[:, i, :], in_=w_d[i * P:(i + 1) * P, :])

            for s in range(cfg.n_spans):
                ts = list(range(s * SPAN, min((s + 1) * SPAN, NT)))
                gpsum = {}
                for t in ts:
                    gpsum[t] = gpsp.tile([P, NS, DT], mybir.dt.float32,
                                         space="PSUM", tag="gp", name="gp")
                for i in range(NS):
                    i_col0 = int(col_of[i, ts[0], 0])
                    i_cols = sum(unit_cols[(s, i, b)] for b in range(NB))
                    if i_cols == 0:
                        continue
                    idx_sb = idxp.tile([P, i_cols * 8], mybir.dt.int16, tag="idx", name="idx")
                    nc.sync.dma_start(out=idx_sb[:], in_=idx_d[:, i_col0 * 8:(i_col0 + i_cols) * 8])
                    dstf_sb = dstfp.tile([P, i_cols, 1], mybir.dt.bfloat16, tag="dstf", name="dstf")
                    nc.sync.dma_start(out=dstf_sb[:, :, 0], in_=dstf_d[:, i_col0:i_col0 + i_cols])
                    s_group = spool.tile([P, i_cols, DT], s_dt, tag="sg", name="sg")
                    nc.vector.tensor_tensor(
                        out=s_group[:],
                        in0=dstf_sb[:].to_broadcast([P, i_cols, DT]),
                        in1=iota_sb[:].to_broadcast([P, i_cols, DT]),
                        op=mybir.AluOpType.is_equal)

                    stg = {}
                    for b in range(NB):
                        ncols = unit_cols[(s, i, b)]
                        if ncols == 0:
                            continue
                        stg[b] = stgp.tile([P, ncols, P], msg_dt, tag=f"stg{b}", name=f"stg{b}")
                        u_col0 = int(col_of[i, ts[0], b])
                        for c0 in range(0, ncols, 96):
                            cw = min(96, ncols - c0)
                            rel16 = (u_col0 - i_col0 + c0) * 8
                            n_idx = cw * P
                            nc.gpsimd.dma_gather(
                                out_ap=stg[b][:, c0:c0 + cw, :],
                                in_ap=x[b * BANK_ROWS:(b + 1) * BANK_ROWS, :],
                                idxs_ap=idx_sb[:, rel16:rel16 + cw * 8],
                                num_idxs=n_idx,
                                num_idxs_reg=n_idx,
                                elem_size=P,
                                single_packet=False,
                                queue_num=gq[0] % 4,
                            )
                            gq[0] += 1
                    for b in range(NB):
                        if b not in stg:
                            continue
                        for t in ts:
                            cc = int(C[i, t, b])
                            if cc == 0:
                                continue
                            gcol0 = int(col_of[i, t, b])
                            rel_s = gcol0 - i_col0
                            rel_b = gcol0 - int(col_of[i, ts[0], b])
                            for k in range(cc):
                                first = (b == _first_b(C, i, t)) and k == 0
                                last = (b == _last_b(C, i, t)) and k == cc - 1
                                nc.tensor.matmul(
                                    out=gpsum[t][:, i, :],
                                    lhsT=stg[b][:, rel_b + k, :],
                                    rhs=s_group[:, rel_s + k, :],
                                    start=first, stop=last)
                for t in ts:
                    gsb = gsbp.tile([P, NS, DT], mybir.dt.float32, tag="g", name="g")
                    for i in range(NS):
                        if C[i, t].sum() == 0:
                            nc.vector.memset(gsb[:, i, :], 0.0)
                        else:
                            nc.vector.tensor_copy(out=gsb[:, i, :], in_=gpsum[t][:, i, :])
                    for half in range(DT // P):
                        d0 = t * DT + half * P
                        rows = min(P, cfg.npc - d0)
                        if rows <= 0:
                            continue
                        opsum = opsp.tile([P, P], mybir.dt.float32, space="PSUM",
                                          tag="o", name="o")
                        for i in range(NS):
                            nc.tensor.matmul(
                                out=opsum[:],
                                lhsT=gsb[:, i, half * P:(half + 1) * P],
                                rhs=w_sb[:, i, :],
                                start=(i == 0), stop=(i == NS - 1))
                        ot = osbp.tile([P, P], mybir.dt.float32, tag="ot", name="ot")
                        nc.vector.tensor_copy(out=ot[:], in_=opsum[:])
                        nc.sync.dma_start(out=out_d[d0:d0 + rows, :], in_=ot[:rows, :])
    nc.compile()
    return nc


def _prepare(hidden_states, edges_i, edges_ii, edges_iii, edges_a,
             W_i, W_ii, W_iii, W_a):
    x = np.asarray(hidden_states, np.float32)
    n_nodes = x.shape[0]
    cfg = _Cfg(n_nodes)
    edges_list = [np.asarray(e) for e in (edges_i, edges_ii, edges_iii, edges_a)]
    w_list = [np.asarray(w, np.float32) for w in (W_i, W_ii, W_iii, W_a)]

    prep = _host_prep(cfg, edges_list)
    nc = _build_kernel(cfg, prep)

    x_pad = np.zeros((cfg.x_rows_pad, P), np.float32)
    x_pad[:n_nodes] = x
    x_pad = x_pad.astype(ml_dtypes.bfloat16)
    iota = np.tile(np.arange(DT_TILE, dtype=np.float32)[None, :], (P, 1)).astype(
        ml_dtypes.bfloat16)
    w_cat = np.concatenate(w_list, axis=0)

    in_maps = [{
        "x": x_pad,
        "idx": prep["idx_mats"][c],
        "dstf": prep["dstf_mats"][c],
        "iota": iota,
        "w": w_cat,
    } for c in range(N_CORES)]
    return nc, in_maps


def kernel(hidden_states, edges_i, edges_ii, edges_iii, edges_a,
           W_i, W_ii, W_iii, W_a):
    nc, in_maps = _prepare(hidden_states, edges_i, edges_ii, edges_iii,
                           edges_a, W_i, W_ii, W_iii, W_a)
    res = run_bass_kernel_spmd(nc, in_maps, core_ids=list(range(N_CORES)))
    out = np.concatenate([res.results[c]["out"] for c in range(N_CORES)], axis=0)
    return out.astype(np.float32)

